# revision 7
# baseline (speedup 1.0000x reference)
"""Dempster-Shafer evidential module on 8 Trainium2 cores.

Math (see v1 notes): the reference's per-step Dempster normalization cancels,
so the scan collapses to an affine recurrence per (batch b, class k):

    z_t = shat[b,t,k]*z_{t-1} + 2/3,   z after prototype 0 = 1 + u[k,0]*rho[b,0]
    shat = 1/3 + (u/3)*rho,  rho = si/(maxsi + 1e-4 - si),  si = exp(T)
    T[p,b] = 2g x.w_p - g|w_p|^2 + ln a - g|x|^2
    y = z_T - 1;  out[b,k] = y/(sum_k y + 1);  out[b,C] = 1/(sum_k y + 1)

v2 structural changes vs v1 (66.3us -> target ~27us):
  - -g|x|^2 is a per-batch-row scalar: computed on host in f64, shipped as an
    f32r (a+b) pair row and added to T by ONE K=2 ones-matmul per quarter.
    This deletes the on-device x=xh+xl add (17us Pool), x^2 square (8.3us
    Act) and the gneg matmul pass (3.4us PE) of v1.
  - per-prototype constant (ln a - g|w|^2) folded into the Exp bias AP
    (exact fp32; deletes 2 K=1 matmuls/quarter).
  - DMA packing: 9 descriptors total (x quarters packed [128,4096]), issued
    x-first so the first T matmul starts at ~2.5us instead of 13.4us
    (HWDGE costs a fixed 625ns per descriptor, serialized).
  - dent on Pool, rho on Pool, rec on DVE; some scans offloaded to Pool and
    some PSUM evacuations to PE (+1/3 via f32r crow2 rows) / kept on Act --
    assignment tuned against the TimelineSim cost model.
  - finals + output DMA per quarter (shorter tail).

Device mapping per core: 2048 batch rows, 4 column-quarters pipelined
(chunk stage lags one quarter); the whole 128-step Dempster fold is ONE
tensor_tensor_scan instruction per 128-row batch chunk (state = data0*state
+ data1, fp32 internal), 10 class segments chained with reset columns.
"""

import numpy as np

B, F, P, C = 16384, 512, 128, 10
NCORES = 8
BL = B // NCORES          # 2048 rows per core
NQ = 4                    # column quarters (512 wide)
NSUB = 4                  # 128-row sub-chunks per quarter
SEG = P + 1               # 129 columns per class segment
QN = C * SEG              # 1290 scan columns
OUTW = 16 * (C + 1)       # 176 packed output columns

# --- tunable schedule (chunk m = 4*q + j, m in 0..15) ---
ONPE = (0, 5, 10)         # chunks whose +1/3 comes from f32r crow2 matmuls on
                          # PE (scan then reads PSUM directly; no Act evac)
POOLSCAN = ()             # (Pool can't run TensorTensorScanArith: real ISA
                          # rejects it even though the cost model prices it)
WR_PASS = True            # include the wr (bf16 residual of w) matmul pass
XL_PASS = True            # include the xl (bf16 residual of x) matmul pass

_PROG = {}
REPS = 1


def _build_program():
    import concourse.bacc as bacc
    import concourse.bass as bass
    import concourse.tile as tile
    from concourse import bass_isa, mybir

    f32 = mybir.dt.float32
    bf16 = mybir.dt.bfloat16
    f32r = mybir.dt.float32r
    Alu = mybir.AluOpType
    Act = mybir.ActivationFunctionType

    nc = bacc.Bacc("TRN2", target_bir_lowering=False, debug=False)

    # x quarters packed: [128, 16384] bf16; quarter q at cols 4096q..,
    # layout | c0..c3 xh (4*512) | c0..c3 xl (4*512) |
    xq_d = nc.dram_tensor("xq", [128, 4 * 4096], bf16, kind="ExternalInput").ap()
    whwr_d = nc.dram_tensor("whwr", [128, 1024], bf16, kind="ExternalInput").ap()
    cb2_d = nc.dram_tensor("cb2", [2, BL], f32r, kind="ExternalInput").ap()
    crowb_d = nc.dram_tensor("crowb", [128, 1], f32, kind="ExternalInput").ap()
    ublk_d = nc.dram_tensor("ublk", [P, QN], f32r, kind="ExternalInput").ap()
    c2ab_d = nc.dram_tensor("crow2ab", [1, 2 * QN], f32r, kind="ExternalInput").ap()
    out_d = nc.dram_tensor("out", [128, OUTW], f32, kind="ExternalOutput").ap()

    with tile.TileContext(nc) as tc:
        for _rep in range(REPS):
            with (
                tc.tile_pool(name="const", bufs=1) as cpool,
                tc.tile_pool(name="xin", bufs=1) as xpool,
                tc.tile_pool(name="mid", bufs=1) as mpool,
                tc.tile_pool(name="scan", bufs=3) as spool,
                tc.tile_pool(name="pst", bufs=2, space=bass.MemorySpace.PSUM) as pst,
                tc.tile_pool(name="pq", bufs=2, space=bass.MemorySpace.PSUM) as pq,
            ):
                # ---- input DMAs, x first (HWDGE serializes at 625ns/desc) ----
                XQ = [xpool.tile([128, 4096], bf16, name=f"xq{q}", tag=f"xq{q}")
                      for q in range(NQ)]
                whwr = cpool.tile([128, 1024], bf16, tag="whwr")
                cb2 = cpool.tile([2, BL], f32r, tag="cb2")
                crowb = cpool.tile([128, 1], f32, tag="crowb")
                ublk = cpool.tile([P, QN], f32r, tag="ublk")
                crow2 = cpool.tile([1, 2 * QN], f32r, tag="crow2")

                nc.sync.dma_start(XQ[0][:], xq_d[:, 0:4096])
                nc.sync.dma_start(whwr[:], whwr_d[:])
                nc.sync.dma_start(cb2[:], cb2_d[:])
                nc.sync.dma_start(crowb[:], crowb_d[:])
                nc.sync.dma_start(XQ[1][:], xq_d[:, 4096:8192])
                nc.sync.dma_start(ublk[:], ublk_d[:])
                nc.sync.dma_start(crow2[:], c2ab_d[:])
                nc.sync.dma_start(XQ[2][:], xq_d[:, 8192:12288])
                nc.sync.dma_start(XQ[3][:], xq_d[:, 12288:16384])

                # ---- device-built constants ----
                ones2f = cpool.tile([2, 128], f32, tag="ones2")
                nc.gpsimd.memset(ones2f[:], 1.0)
                ones2 = ones2f[:].bitcast(f32r)
                data1 = cpool.tile([128, QN], f32, tag="data1")
                nc.gpsimd.memset(data1[:], 2.0 / 3.0)
                d1v = data1[:].rearrange("p (k s) -> p k s", s=SEG)
                nc.gpsimd.memset(d1v[:, :, 0], 1.0)

                zf = mpool.tile([128, 16 * C], f32, tag="zf")
                nsplit = [(0, 512), (512, 512), (1024, QN - 1024)]
                rho_q = []

                # ---- finals for one quarter: y=z-1, dr=1/(sum z - 9),
                #      out = z*dr - dr, out[C] = dr ----
                def q_finals(q):
                    szq = mpool.tile([128, NSUB], f32, name=f"sz{q}", tag="szq",
                                     bufs=2)
                    nc.vector.tensor_reduce(
                        szq[:],
                        zf[:, 40 * q:40 * (q + 1)].rearrange(
                            "p (s k) -> p s k", k=C),
                        axis=mybir.AxisListType.X, op=Alu.add)
                    nc.vector.tensor_scalar_add(szq[:], szq[:], -(C - 1.0))
                    drq = mpool.tile([128, NSUB], f32, name=f"dr{q}", tag="drq",
                                     bufs=2)
                    nc.vector.reciprocal(drq[:], szq[:])
                    outq = mpool.tile([128, NSUB * (C + 1)], f32,
                                      name=f"outq{q}", tag="outq", bufs=2)
                    for s in range(NSUB):
                        m = 4 * q + s
                        nc.vector.tensor_scalar(
                            outq[:, (C + 1) * s:(C + 1) * s + C],
                            zf[:, C * m:C * (m + 1)],
                            scalar1=drq[:, s:s + 1], scalar2=drq[:, s:s + 1],
                            op0=Alu.mult, op1=Alu.subtract)
                    ov = outq[:].rearrange("p (s k) -> p s k", k=C + 1)
                    nc.gpsimd.tensor_copy(ov[:, :, C], drq[:])
                    nc.sync.dma_start(
                        out_d[:, 44 * q:44 * (q + 1)], outq[:])

                # ---- chunk stage for one quarter (lags one quarter) ----
                def q_stage(q):
                    rho = rho_q[q]
                    for j in range(NSUB):
                        m = 4 * q + j
                        on_pe = m in ONPE
                        qs = pq.tile([128, QN], f32, name=f"qs{m}", tag="qs")
                        for (o, n) in nsplit:
                            nc.tensor.matmul(
                                qs[:, o:o + n], rho[:, 128 * j:128 * (j + 1)],
                                ublk[:, o:o + n], start=True, stop=not on_pe)
                            if on_pe:
                                nc.tensor.matmul(
                                    qs[:, o:o + n], ones2[0:1, :],
                                    crow2[:, o:o + n],
                                    start=False, stop=False)
                                nc.tensor.matmul(
                                    qs[:, o:o + n], ones2[0:1, :],
                                    crow2[:, QN + o:QN + o + n],
                                    start=False, stop=True)
                        so = spool.tile([128, QN], f32, name=f"so{m}", tag="so")
                        if on_pe:
                            data0 = qs
                        else:
                            sh = spool.tile([128, QN], f32, name=f"sh{m}",
                                            tag="sh")
                            nc.scalar.activation(sh[:], qs[:], Act.Copy,
                                                 bias=1.0 / 3.0)
                            data0 = sh
                        eng = nc.gpsimd if m in POOLSCAN else nc.vector
                        eng.tensor_tensor_scan(
                            so[:], data0[:], data1[:], initial=1.0,
                            op0=Alu.mult, op1=Alu.add)
                        sov = so[:].rearrange("p (k s) -> p k s", s=SEG)
                        nc.gpsimd.tensor_copy(
                            zf[:, C * m:C * (m + 1)], sov[:, :, SEG - 1])
                    q_finals(q)

                # ---- per column-quarter pipeline ----
                for q in range(NQ):
                    cs = 512 * q
                    T = pst.tile([128, 512], f32, tag="T")
                    first = True
                    for c in range(4):     # wh . xh
                        nc.tensor.matmul(
                            T[:], whwr[:, 128 * c:128 * (c + 1)],
                            XQ[q][:, 512 * c:512 * (c + 1)],
                            start=first, stop=False)
                        first = False
                    if XL_PASS:
                        for c in range(4):  # wh . xl
                            nc.tensor.matmul(
                                T[:], whwr[:, 128 * c:128 * (c + 1)],
                                XQ[q][:, 2048 + 512 * c:2048 + 512 * (c + 1)],
                                start=False, stop=False)
                    if WR_PASS:
                        for c in range(4):  # wr . xh
                            nc.tensor.matmul(
                                T[:], whwr[:, 512 + 128 * c:512 + 128 * (c + 1)],
                                XQ[q][:, 512 * c:512 * (c + 1)],
                                start=False, stop=False)
                    # -g|x|^2 (f32r a+b rows, summed by a K=2 ones matmul)
                    nc.tensor.matmul(T[:], ones2, cb2[:, cs:cs + 512],
                                     start=False, stop=True)

                    si = mpool.tile([128, 512], f32, name=f"si{q}", tag="si",
                                    bufs=3)
                    nc.scalar.activation(si[:], T[:], Act.Exp,
                                         bias=crowb[:, 0:1])
                    amax = mpool.tile([128, 512], f32, name=f"amax{q}",
                                      tag="amax", bufs=2)
                    nc.gpsimd.partition_all_reduce(
                        amax[:], si[:], channels=128,
                        reduce_op=bass_isa.ReduceOp.max)
                    dent = spool.tile([128, 512], f32, name=f"dent{q}",
                                      tag="dent")
                    nc.vector.scalar_tensor_tensor(
                        dent[:], amax[:], 1e-4, si[:],
                        op0=Alu.add, op1=Alu.subtract)
                    rec = mpool.tile([128, 512], f32, name=f"rec{q}",
                                     tag="rec", bufs=2)
                    nc.vector.reciprocal_approx_fast(rec[:], dent[:])
                    rho = mpool.tile([128, 512], f32r, name=f"rho{q}",
                                     tag="rho", bufs=4)
                    nc.gpsimd.tensor_mul(rho[:], si[:], rec[:])
                    rho_q.append(rho)
                    if q >= 1:
                        q_stage(q - 1)
                q_stage(NQ - 1)

    nc.compile()
    return nc


def _f32r_round(v):
    # float32r = RNE to 11 explicit mantissa bits (HW-verified).
    u = np.asarray(v, np.float32).view(np.uint32).astype(np.uint64)
    drop = 12
    half = np.uint64(1 << (drop - 1))
    odd = (u >> np.uint64(drop)) & np.uint64(1)
    u2 = (u + half - np.uint64(1) + odd) & np.uint64(~((1 << drop) - 1) & 0xFFFFFFFF)
    return u2.astype(np.uint32).view(np.float32)


def _host_prep(x, w, eta, xi, beta):
    """Host-side: shard/layout x, build tiny replicated param matrices."""
    import ml_dtypes

    x = np.asarray(x, np.float32)
    w = np.asarray(w, np.float32)
    eta = np.asarray(eta, np.float32).reshape(-1)
    xi = np.asarray(xi, np.float32).reshape(-1)
    beta = np.asarray(beta, np.float32)

    gamma = (eta.astype(np.float64)) ** 2                # [P]
    if np.ptp(gamma) != 0.0:
        raise NotImplementedError(
            "kernel assumes per-prototype-constant gamma (eta); the shipped "
            "problem uses eta = full(0.1)")
    g0 = float(gamma[0])
    alpha = 1.0 / (1.0 + np.exp(-xi.astype(np.float64)))
    wsq = (w.astype(np.float64) ** 2).sum(-1)            # [P]

    wt2g = (2.0 * gamma[None, :] * w.T.astype(np.float64)).astype(np.float32)  # [F,P]
    wh = wt2g.astype(ml_dtypes.bfloat16)
    wr = (wt2g.astype(np.float64) - wh.astype(np.float64)).astype(np.float32).astype(ml_dtypes.bfloat16)
    # packed [128, 1024]: wh chunks c0..3 then wr chunks c0..3
    whwr = np.zeros((128, 1024), ml_dtypes.bfloat16)
    for c in range(4):
        whwr[:, 128 * c:128 * (c + 1)] = wh[128 * c:128 * (c + 1), :]
        whwr[:, 512 + 128 * c:512 + 128 * (c + 1)] = wr[128 * c:128 * (c + 1), :]

    crow_bias = (np.log(alpha) - gamma * wsq).astype(np.float32)[:, None]  # [P,1]

    b2 = beta.astype(np.float64) ** 2
    u = b2 / b2.sum(0, keepdims=True)                    # [C,P]
    uh = u / 3.0
    third_a = float(_f32r_round(np.float32(1.0 / 3.0)))
    third_b = np.float32(1.0 / 3.0 - third_a)
    ublk = np.zeros((P, QN), np.float32)
    crow2ab = np.zeros((1, 2 * QN), np.float32)
    for k in range(C):
        base = k * SEG
        crow2ab[0, base + 1:base + SEG] = third_a
        crow2ab[0, QN + base + 1:QN + base + SEG] = third_b
        for t in range(P):
            v = uh[k, t] * (3.0 if t == 0 else 1.0)
            ublk[t, base + 1 + t] = np.float32(v)

    shards = x.reshape(NCORES, BL, F)
    in_maps = []
    for i in range(NCORES):
        xs = shards[i]                                    # [BL, F] f32
        xt = np.ascontiguousarray(xs.T)                   # [F, BL]
        xh = xt.astype(ml_dtypes.bfloat16)
        xl = (xt.astype(np.float64) - xh.astype(np.float64)).astype(np.float32).astype(ml_dtypes.bfloat16)
        xqp = np.zeros((128, 4 * 4096), ml_dtypes.bfloat16)
        for q in range(4):
            for c in range(4):
                xqp[:, 4096 * q + 512 * c:4096 * q + 512 * (c + 1)] = \
                    xh[128 * c:128 * (c + 1), 512 * q:512 * (q + 1)]
                xqp[:, 4096 * q + 2048 + 512 * c:4096 * q + 2048 + 512 * (c + 1)] = \
                    xl[128 * c:128 * (c + 1), 512 * q:512 * (q + 1)]
        # -g|x|^2 per batch row, f64 -> f32r a + f32r b residual rows
        cb = -(g0 * (xs.astype(np.float64) ** 2).sum(-1))          # [BL]
        cba = _f32r_round(cb.astype(np.float32))
        cbb = _f32r_round((cb - cba.astype(np.float64)).astype(np.float32))
        cb2 = np.stack([cba, cbb], axis=0)                         # [2, BL]
        in_maps.append({
            "xq": xqp, "whwr": whwr, "cb2": cb2, "crowb": crow_bias,
            "ublk": ublk, "crow2ab": crow2ab,
        })
    return in_maps


def _run(in_maps, trace=False):
    from concourse.bass_utils import run_bass_kernel_spmd

    if "nc" not in _PROG:
        _PROG["nc"] = _build_program()
    nc = _PROG["nc"]
    res = run_bass_kernel_spmd(
        nc, in_maps, core_ids=list(range(NCORES)), trace=trace)
    outs = []
    for i in range(NCORES):
        o = np.asarray(res.results[i]["out"])          # [128, 176]
        outs.append(o.reshape(128, 16, C + 1).transpose(1, 0, 2).reshape(BL, C + 1))
    full = np.concatenate(outs, axis=0).astype(np.float32)
    return full, res


def kernel(x, w, eta, xi, beta):
    in_maps = _host_prep(x, w, eta, xi, beta)
    full, _ = _run(in_maps, trace=False)
    return full


# revision 9
# speedup vs baseline: 1.0913x; 1.0913x over previous
"""Dempster-Shafer evidential module on 8 Trainium2 cores.

Math (see v1 notes): the reference's per-step Dempster normalization cancels,
so the scan collapses to an affine recurrence per (batch b, class k):

    z_t = shat[b,t,k]*z_{t-1} + 2/3,   z after prototype 0 = 1 + u[k,0]*rho[b,0]
    shat = 1/3 + (u/3)*rho,  rho = si/(maxsi + 1e-4 - si),  si = exp(T)
    T[p,b] = 2g x.w_p - g|w_p|^2 + ln a - g|x|^2
    y = z_T - 1;  out[b,k] = y/(sum_k y + 1);  out[b,C] = 1/(sum_k y + 1)

v2 structural changes vs v1 (66.3us -> target ~27us):
  - -g|x|^2 is a per-batch-row scalar: computed on host in f64, shipped as an
    f32r (a+b) pair row and added to T by ONE K=2 ones-matmul per quarter.
    This deletes the on-device x=xh+xl add (17us Pool), x^2 square (8.3us
    Act) and the gneg matmul pass (3.4us PE) of v1.
  - per-prototype constant (ln a - g|w|^2) folded into the Exp bias AP
    (exact fp32; deletes 2 K=1 matmuls/quarter).
  - DMA packing: 9 descriptors total (x quarters packed [128,4096]), issued
    x-first so the first T matmul starts at ~2.5us instead of 13.4us
    (HWDGE costs a fixed 625ns per descriptor, serialized).
  - dent on Pool, rho on Pool, rec on DVE; some scans offloaded to Pool and
    some PSUM evacuations to PE (+1/3 via f32r crow2 rows) / kept on Act --
    assignment tuned against the TimelineSim cost model.
  - finals + output DMA per quarter (shorter tail).

Device mapping per core: 2048 batch rows, 4 column-quarters pipelined
(chunk stage lags one quarter); the whole 128-step Dempster fold is ONE
tensor_tensor_scan instruction per 128-row batch chunk (state = data0*state
+ data1, fp32 internal), 10 class segments chained with reset columns.
"""

import numpy as np

B, F, P, C = 16384, 512, 128, 10
NCORES = 8
BL = B // NCORES          # 2048 rows per core
NQ = 4                    # column quarters (512 wide)
NSUB = 4                  # 128-row sub-chunks per quarter
SEG = P + 1               # 129 columns per class segment
QN = C * SEG              # 1290 scan columns
OUTW = 16 * (C + 1)       # 176 packed output columns

# --- tunable schedule (chunk m = 4*q + j, m in 0..15) ---
ONPE = (5, 10)            # chunks whose +1/3 comes from f32r crow2 matmuls on
                          # PE (scan then reads PSUM directly; no Act evac)
POOLSCAN = ()             # (Pool can't run TensorTensorScanArith: real ISA
                          # rejects it even though the cost model prices it)
WR_PASS = True            # include the wr (bf16 residual of w) matmul pass
XL_PASS = True            # include the xl (bf16 residual of x) matmul pass

WARMUP = 14               # PE p-state warm-up matmuls

_PROG = {}
REPS = 1


def _build_program():
    import concourse.bacc as bacc
    import concourse.bass as bass
    import concourse.tile as tile
    from concourse import bass_isa, mybir

    f32 = mybir.dt.float32
    bf16 = mybir.dt.bfloat16
    f32r = mybir.dt.float32r
    Alu = mybir.AluOpType
    Act = mybir.ActivationFunctionType

    nc = bacc.Bacc("TRN2", target_bir_lowering=False, debug=False)

    # x quarters packed: [128, 16384] bf16; quarter q at cols 4096q..,
    # layout | c0..c3 xh (4*512) | c0..c3 xl (4*512) |
    xq_d = nc.dram_tensor("xq", [128, 4 * 4096], bf16, kind="ExternalInput").ap()
    whwr_d = nc.dram_tensor("whwr", [128, 1024], bf16, kind="ExternalInput").ap()
    cb2_d = nc.dram_tensor("cb2", [2, BL], f32r, kind="ExternalInput").ap()
    crowb_d = nc.dram_tensor("crowb", [128, 1], f32, kind="ExternalInput").ap()
    ublk_d = nc.dram_tensor("ublk", [P, QN], f32r, kind="ExternalInput").ap()
    c2ab_d = nc.dram_tensor("crow2ab", [1, 2 * QN], f32r, kind="ExternalInput").ap()
    out_d = nc.dram_tensor("out", [128, OUTW], f32, kind="ExternalOutput").ap()

    with tile.TileContext(nc) as tc:
        for _rep in range(REPS):
            with (
                tc.tile_pool(name="const", bufs=1) as cpool,
                tc.tile_pool(name="xin", bufs=1) as xpool,
                tc.tile_pool(name="mid", bufs=1) as mpool,
                tc.tile_pool(name="scan", bufs=3) as spool,
                tc.tile_pool(name="pst", bufs=2, space=bass.MemorySpace.PSUM) as pst,
                tc.tile_pool(name="pq", bufs=2, space=bass.MemorySpace.PSUM) as pq,
            ):
                # ---- input DMAs, x first (HWDGE serializes at 625ns/desc) ----
                XQ = [xpool.tile([128, 4096], bf16, name=f"xq{q}", tag=f"xq{q}")
                      for q in range(NQ)]
                whwr = cpool.tile([128, 1024], bf16, tag="whwr")
                cb2 = cpool.tile([2, BL], f32r, tag="cb2")
                crowb = cpool.tile([128, 1], f32, tag="crowb")
                ublk = cpool.tile([P, QN], f32r, tag="ublk")
                crow2 = cpool.tile([1, 2 * QN], f32r, tag="crow2")

                nc.sync.dma_start(whwr[:], whwr_d[:])
                nc.sync.dma_start(XQ[0][:, 0:2048], xq_d[:, 0:2048])
                nc.sync.dma_start(cb2[:], cb2_d[:])
                nc.sync.dma_start(crowb[:], crowb_d[:])
                nc.sync.dma_start(XQ[0][:, 2048:4096], xq_d[:, 2048:4096])
                nc.sync.dma_start(XQ[1][:], xq_d[:, 4096:8192])
                nc.sync.dma_start(ublk[:], ublk_d[:])
                nc.sync.dma_start(crow2[:], c2ab_d[:])
                nc.sync.dma_start(XQ[2][:], xq_d[:, 8192:12288])
                nc.sync.dma_start(XQ[3][:], xq_d[:, 12288:16384])

                # ---- device-built constants ----
                ones2f = cpool.tile([2, 128], f32, tag="ones2")
                nc.gpsimd.memset(ones2f[:], 1.0)
                ones2 = ones2f[:].bitcast(f32r)
                warmb = cpool.tile([2, 256], bf16, tag="warmb")
                nc.gpsimd.memset(warmb[:], 1.0)
                wpsum = pst.tile([128, 512], f32, name="warm", tag="T")
                for _wi in range(WARMUP):
                    nc.tensor.matmul(wpsum[:, 0:128], warmb[:, 0:128],
                                     warmb[:, 128:256], start=True, stop=True)
                data1 = cpool.tile([128, QN], f32, tag="data1")
                nc.gpsimd.memset(data1[:], 2.0 / 3.0)
                d1v = data1[:].rearrange("p (k s) -> p k s", s=SEG)
                nc.gpsimd.memset(d1v[:, :, 0], 1.0)

                zf = mpool.tile([128, 16 * C], f32, tag="zf")
                nsplit = [(0, 512), (512, 512), (1024, QN - 1024)]
                rho_q = []

                # ---- finals for one quarter: y=z-1, dr=1/(sum z - 9),
                #      out = z*dr - dr, out[C] = dr ----
                def q_finals(q):
                    szq = mpool.tile([128, NSUB], f32, name=f"sz{q}", tag="szq",
                                     bufs=2)
                    nc.vector.tensor_reduce(
                        szq[:],
                        zf[:, 40 * q:40 * (q + 1)].rearrange(
                            "p (s k) -> p s k", k=C),
                        axis=mybir.AxisListType.X, op=Alu.add)
                    nc.vector.tensor_scalar_add(szq[:], szq[:], -(C - 1.0))
                    drq = mpool.tile([128, NSUB], f32, name=f"dr{q}", tag="drq",
                                     bufs=2)
                    nc.vector.reciprocal(drq[:], szq[:])
                    outq = mpool.tile([128, NSUB * (C + 1)], f32,
                                      name=f"outq{q}", tag="outq", bufs=2)
                    for s in range(NSUB):
                        m = 4 * q + s
                        nc.vector.tensor_scalar(
                            outq[:, (C + 1) * s:(C + 1) * s + C],
                            zf[:, C * m:C * (m + 1)],
                            scalar1=drq[:, s:s + 1], scalar2=drq[:, s:s + 1],
                            op0=Alu.mult, op1=Alu.subtract)
                    ov = outq[:].rearrange("p (s k) -> p s k", k=C + 1)
                    nc.gpsimd.tensor_copy(ov[:, :, C], drq[:])
                    nc.sync.dma_start(
                        out_d[:, 44 * q:44 * (q + 1)], outq[:])

                # ---- chunk stage for one quarter (lags one quarter) ----
                def q_stage(q):
                    rho = rho_q[q]
                    for j in range(NSUB):
                        m = 4 * q + j
                        on_pe = m in ONPE
                        qs = pq.tile([128, QN], f32, name=f"qs{m}", tag="qs")
                        for (o, n) in nsplit:
                            nc.tensor.matmul(
                                qs[:, o:o + n], rho[:, 128 * j:128 * (j + 1)],
                                ublk[:, o:o + n], start=True, stop=not on_pe)
                            if on_pe:
                                nc.tensor.matmul(
                                    qs[:, o:o + n], ones2[0:1, :],
                                    crow2[:, o:o + n],
                                    start=False, stop=False)
                                nc.tensor.matmul(
                                    qs[:, o:o + n], ones2[0:1, :],
                                    crow2[:, QN + o:QN + o + n],
                                    start=False, stop=True)
                        so = spool.tile([128, QN], f32, name=f"so{m}", tag="so")
                        if on_pe:
                            data0 = qs
                        else:
                            sh = spool.tile([128, QN], f32, name=f"sh{m}",
                                            tag="sh")
                            nc.scalar.activation(sh[:], qs[:], Act.Copy,
                                                 bias=1.0 / 3.0)
                            data0 = sh
                        eng = nc.gpsimd if m in POOLSCAN else nc.vector
                        eng.tensor_tensor_scan(
                            so[:], data0[:], data1[:], initial=1.0,
                            op0=Alu.mult, op1=Alu.add)
                        sov = so[:].rearrange("p (k s) -> p k s", s=SEG)
                        nc.gpsimd.tensor_copy(
                            zf[:, C * m:C * (m + 1)], sov[:, :, SEG - 1])
                    q_finals(q)

                # ---- per column-quarter pipeline ----
                for q in range(NQ):
                    cs = 512 * q
                    T = pst.tile([128, 512], f32, tag="T")
                    first = True
                    for c in range(4):     # wh . xh
                        nc.tensor.matmul(
                            T[:], whwr[:, 128 * c:128 * (c + 1)],
                            XQ[q][:, 512 * c:512 * (c + 1)],
                            start=first, stop=False)
                        first = False
                    if XL_PASS:
                        for c in range(4):  # wh . xl
                            nc.tensor.matmul(
                                T[:], whwr[:, 128 * c:128 * (c + 1)],
                                XQ[q][:, 2048 + 512 * c:2048 + 512 * (c + 1)],
                                start=False, stop=False)
                    if WR_PASS:
                        for c in range(4):  # wr . xh
                            nc.tensor.matmul(
                                T[:], whwr[:, 512 + 128 * c:512 + 128 * (c + 1)],
                                XQ[q][:, 512 * c:512 * (c + 1)],
                                start=False, stop=False)
                    # -g|x|^2 (f32r a+b rows, summed by a K=2 ones matmul)
                    nc.tensor.matmul(T[:], ones2, cb2[:, cs:cs + 512],
                                     start=False, stop=True)

                    si = mpool.tile([128, 512], f32, name=f"si{q}", tag="si",
                                    bufs=3)
                    nc.scalar.activation(si[:], T[:], Act.Exp,
                                         bias=crowb[:, 0:1])
                    amax = mpool.tile([128, 512], f32, name=f"amax{q}",
                                      tag="amax", bufs=2)
                    nc.gpsimd.partition_all_reduce(
                        amax[:], si[:], channels=128,
                        reduce_op=bass_isa.ReduceOp.max)
                    d0 = spool.tile([128, 512], f32, name=f"d0{q}", tag="d0")
                    nc.gpsimd.tensor_sub(d0[:], amax[:], si[:])
                    dent = spool.tile([128, 512], f32, name=f"dent{q}",
                                      tag="dent")
                    nc.scalar.activation(dent[:], d0[:], Act.Copy, bias=1e-4)
                    rec = mpool.tile([128, 512], f32, name=f"rec{q}",
                                     tag="rec", bufs=2)
                    nc.vector.reciprocal_approx_fast(rec[:], dent[:])
                    rho = mpool.tile([128, 512], f32r, name=f"rho{q}",
                                     tag="rho", bufs=4)
                    nc.gpsimd.tensor_mul(rho[:], si[:], rec[:])
                    rho_q.append(rho)
                    if q >= 1:
                        q_stage(q - 1)
                q_stage(NQ - 1)

    nc.compile()
    return nc


def _f32r_round(v):
    # float32r = RNE to 11 explicit mantissa bits (HW-verified).
    u = np.asarray(v, np.float32).view(np.uint32).astype(np.uint64)
    drop = 12
    half = np.uint64(1 << (drop - 1))
    odd = (u >> np.uint64(drop)) & np.uint64(1)
    u2 = (u + half - np.uint64(1) + odd) & np.uint64(~((1 << drop) - 1) & 0xFFFFFFFF)
    return u2.astype(np.uint32).view(np.float32)


def _host_prep(x, w, eta, xi, beta):
    """Host-side: shard/layout x, build tiny replicated param matrices."""
    import ml_dtypes

    x = np.asarray(x, np.float32)
    w = np.asarray(w, np.float32)
    eta = np.asarray(eta, np.float32).reshape(-1)
    xi = np.asarray(xi, np.float32).reshape(-1)
    beta = np.asarray(beta, np.float32)

    gamma = (eta.astype(np.float64)) ** 2                # [P]
    if np.ptp(gamma) != 0.0:
        raise NotImplementedError(
            "kernel assumes per-prototype-constant gamma (eta); the shipped "
            "problem uses eta = full(0.1)")
    g0 = float(gamma[0])
    alpha = 1.0 / (1.0 + np.exp(-xi.astype(np.float64)))
    wsq = (w.astype(np.float64) ** 2).sum(-1)            # [P]

    wt2g = (2.0 * gamma[None, :] * w.T.astype(np.float64)).astype(np.float32)  # [F,P]
    wh = wt2g.astype(ml_dtypes.bfloat16)
    wr = (wt2g.astype(np.float64) - wh.astype(np.float64)).astype(np.float32).astype(ml_dtypes.bfloat16)
    # packed [128, 1024]: wh chunks c0..3 then wr chunks c0..3
    whwr = np.zeros((128, 1024), ml_dtypes.bfloat16)
    for c in range(4):
        whwr[:, 128 * c:128 * (c + 1)] = wh[128 * c:128 * (c + 1), :]
        whwr[:, 512 + 128 * c:512 + 128 * (c + 1)] = wr[128 * c:128 * (c + 1), :]

    crow_bias = (np.log(alpha) - gamma * wsq).astype(np.float32)[:, None]  # [P,1]

    b2 = beta.astype(np.float64) ** 2
    u = b2 / b2.sum(0, keepdims=True)                    # [C,P]
    uh = u / 3.0
    third_a = float(_f32r_round(np.float32(1.0 / 3.0)))
    third_b = np.float32(1.0 / 3.0 - third_a)
    ublk = np.zeros((P, QN), np.float32)
    crow2ab = np.zeros((1, 2 * QN), np.float32)
    for k in range(C):
        base = k * SEG
        crow2ab[0, base + 1:base + SEG] = third_a
        crow2ab[0, QN + base + 1:QN + base + SEG] = third_b
        for t in range(P):
            v = uh[k, t] * (3.0 if t == 0 else 1.0)
            ublk[t, base + 1 + t] = np.float32(v)

    shards = x.reshape(NCORES, BL, F)
    in_maps = []
    for i in range(NCORES):
        xs = shards[i]                                    # [BL, F] f32
        xt = np.ascontiguousarray(xs.T)                   # [F, BL]
        xh = xt.astype(ml_dtypes.bfloat16)
        xl = (xt.astype(np.float64) - xh.astype(np.float64)).astype(np.float32).astype(ml_dtypes.bfloat16)
        xqp = np.zeros((128, 4 * 4096), ml_dtypes.bfloat16)
        for q in range(4):
            for c in range(4):
                xqp[:, 4096 * q + 512 * c:4096 * q + 512 * (c + 1)] = \
                    xh[128 * c:128 * (c + 1), 512 * q:512 * (q + 1)]
                xqp[:, 4096 * q + 2048 + 512 * c:4096 * q + 2048 + 512 * (c + 1)] = \
                    xl[128 * c:128 * (c + 1), 512 * q:512 * (q + 1)]
        # -g|x|^2 per batch row, f64 -> f32r a + f32r b residual rows
        cb = -(g0 * (xs.astype(np.float64) ** 2).sum(-1))          # [BL]
        cba = _f32r_round(cb.astype(np.float32))
        cbb = _f32r_round((cb - cba.astype(np.float64)).astype(np.float32))
        cb2 = np.stack([cba, cbb], axis=0)                         # [2, BL]
        in_maps.append({
            "xq": xqp, "whwr": whwr, "cb2": cb2, "crowb": crow_bias,
            "ublk": ublk, "crow2ab": crow2ab,
        })
    return in_maps


def _run(in_maps, trace=False):
    from concourse.bass_utils import run_bass_kernel_spmd

    if "nc" not in _PROG:
        _PROG["nc"] = _build_program()
    nc = _PROG["nc"]
    res = run_bass_kernel_spmd(
        nc, in_maps, core_ids=list(range(NCORES)), trace=trace)
    outs = []
    for i in range(NCORES):
        o = np.asarray(res.results[i]["out"])          # [128, 176]
        outs.append(o.reshape(128, 16, C + 1).transpose(1, 0, 2).reshape(BL, C + 1))
    full = np.concatenate(outs, axis=0).astype(np.float32)
    return full, res


def kernel(x, w, eta, xi, beta):
    in_maps = _host_prep(x, w, eta, xi, beta)
    full, _ = _run(in_maps, trace=False)
    return full


# revision 13
# speedup vs baseline: 1.1591x; 1.0622x over previous
"""Dempster-Shafer evidential module on 8 Trainium2 cores.

Math: the reference's per-step Dempster normalization cancels, so the scan
collapses to an affine recurrence per (batch b, class k):

    z_t = shat[b,t,k]*z_{t-1} + 2/3,   z after prototype 0 = 1 + u[k,0]*rho[b,0]
    shat = 1/3 + (u/3)*rho,  rho = si/(maxsi + 1e-4 - si),  si = exp(T)
    T[p,b] = 2g x.w_p - g|w_p|^2 + ln a - g|x|^2
    y = z_T - 1;  out[b,k] = y/(sum_k y + 1);  out[b,C] = 1/(sum_k y + 1)

Key structural points (v3; v1 was 66.3us, v2 51.0us):
  - -g|x|^2 is a per-batch-row scalar: computed on host in f64, shipped as an
    f32r (a+b) pair row and added to T by ONE K=2 ones-matmul per slice.
  - per-prototype constant (ln a - g|w|^2) folded into the Exp bias AP.
  - x shipped bf16-only (no hi/lo split): with |x|^2 exact from the host, the
    x.w term tolerates bf16 quantization (the exponent error ~1e-4 amplifies
    ~50x at the cancellation-amplified argmax but stays well under the 2e-2
    gate). Same for w. Halves x DMA bytes and T matmul passes.
  - batch processed in column SLICES: quarter 0 split into 4x128-col slices
    so the first scan starts ~7us instead of ~16us; rest 512-wide. Host x
    layout is slice-contiguous ([slice][c] 128-col blocks) so any slice is a
    contiguous DMA range. The chunk stage (qs matmul -> evac -> 1290-col
    Dempster scan on DVE) lags one slice.
  - rho = si / dent via gpsimd divide on Pool, dent = (amax - si) on Pool
    + 1e-4 bias on Act: keeps DVE (the scan engine, the critical resource)
    free of everything except scans + small finals.
  - PE p-state warm-up (dummy matmuls) so real matmuls start at full clock.
  - finals per quarter; per chunk on the last quarter to shorten the tail.
"""

import numpy as np

B, F, P, C = 16384, 512, 128, 10
NCORES = 8
BL = B // NCORES          # 2048 rows per core
SEG = P + 1               # 129 columns per class segment
QN = C * SEG              # 1290 scan columns
OUTW = 16 * (C + 1)       # 176 packed output columns

# batch-column slices (start, ncols); chunk m = col/128, 16 chunks total
SLICES = [(0, 128), (128, 128), (256, 128), (384, 128),
          (512, 512), (1024, 512), (1536, 512)]
ONPE = (5, 8, 11, 14)     # chunks whose +1/3 comes from f32r crow2 matmuls on
                          # PE (scan then reads PSUM directly; no Act evac)
WARMUP = 14               # PE p-state warm-up matmuls

_PROG = {}
REPS = 1


def _build_program():
    import concourse.bacc as bacc
    import concourse.bass as bass
    import concourse.tile as tile
    from concourse import bass_isa, mybir

    f32 = mybir.dt.float32
    bf16 = mybir.dt.bfloat16
    f16 = mybir.dt.float16
    f32r = mybir.dt.float32r
    Alu = mybir.AluOpType
    Act = mybir.ActivationFunctionType

    nc = bacc.Bacc("TRN2", target_bir_lowering=False, debug=False)

    # x slice-contiguous: 128-col block (slice s, chunk c) at col s*512+c*128
    xq_d = nc.dram_tensor("xq", [128, 8192], f16, kind="ExternalInput").ap()
    wh_d = nc.dram_tensor("whp", [128, 1024], f16, kind="ExternalInput").ap()
    cb2_d = nc.dram_tensor("cb2", [2, BL], f32r, kind="ExternalInput").ap()
    crowb_d = nc.dram_tensor("crowb", [128, 1], f32, kind="ExternalInput").ap()
    ublk_d = nc.dram_tensor("ublk", [P, QN], f32r, kind="ExternalInput").ap()
    c2ab_d = nc.dram_tensor("crow2ab", [1, 2 * QN], f32r, kind="ExternalInput").ap()
    out_d = nc.dram_tensor("out", [128, OUTW], f32, kind="ExternalOutput").ap()

    with tile.TileContext(nc) as tc:
        for _rep in range(REPS):
            with (
                tc.tile_pool(name="const", bufs=1) as cpool,
                tc.tile_pool(name="xin", bufs=1) as xpool,
                tc.tile_pool(name="mid", bufs=1) as mpool,
                tc.tile_pool(name="scan", bufs=3) as spool,
                tc.tile_pool(name="pst", bufs=2, space=bass.MemorySpace.PSUM) as pst,
                tc.tile_pool(name="pq", bufs=2, space=bass.MemorySpace.PSUM) as pq,
            ):
                # ---- input DMAs (HWDGE serializes at 625ns/descriptor; DMA
                #      transfers serialize at ~360B/ns: order = startup path) ----
                xall = xpool.tile([128, 8192], f16, tag="xall")
                wh = cpool.tile([128, 1024], f16, tag="wh")
                cb2 = cpool.tile([2, BL], f32r, tag="cb2")
                crowb = cpool.tile([128, 1], f32, tag="crowb")
                ublk = cpool.tile([P, QN], f32r, tag="ublk")
                crow2 = cpool.tile([1, 2 * QN], f32r, tag="crow2")

                nc.sync.dma_start(wh[:], wh_d[:])
                nc.sync.dma_start(cb2[:], cb2_d[:])
                nc.sync.dma_start(xall[:, 0:512], xq_d[:, 0:512])
                nc.sync.dma_start(crowb[:], crowb_d[:])
                nc.sync.dma_start(xall[:, 512:1024], xq_d[:, 512:1024])
                nc.sync.dma_start(xall[:, 1024:2048], xq_d[:, 1024:2048])
                nc.sync.dma_start(ublk[:], ublk_d[:])
                nc.sync.dma_start(xall[:, 2048:4096], xq_d[:, 2048:4096])
                nc.sync.dma_start(crow2[:], c2ab_d[:])
                nc.sync.dma_start(xall[:, 4096:6144], xq_d[:, 4096:6144])
                nc.sync.dma_start(xall[:, 6144:8192], xq_d[:, 6144:8192])

                # ---- device-built constants + PE warm-up ----
                ones2f = cpool.tile([2, 128], f32, tag="ones2")
                nc.gpsimd.memset(ones2f[:], 1.0)
                ones2 = ones2f[:].bitcast(f32r)
                warmb = cpool.tile([2, 256], bf16, tag="warmb")
                nc.gpsimd.memset(warmb[:], 1.0)
                wpsum = pst.tile([128, 512], f32, name="warm", tag="T")
                for _wi in range(WARMUP):
                    nc.tensor.matmul(wpsum[:, 0:128], warmb[:, 0:128],
                                     warmb[:, 128:256], start=True, stop=True)
                data1 = cpool.tile([128, QN], f32, tag="data1")
                nc.gpsimd.memset(data1[:], 2.0 / 3.0)
                d1v = data1[:].rearrange("p (k s) -> p k s", s=SEG)
                nc.gpsimd.memset(d1v[:, :, 0], 1.0)

                zf = mpool.tile([128, 16 * C], f32, tag="zf")
                nsplit = [(0, 512), (512, 512), (1024, QN - 1024)]

                def _xslice(cs, w, c):
                    # x cols for chunk-range [cs, cs+w), weight chunk c
                    s0, n = cs // 128, w // 128
                    v = xall[:].rearrange("p (s c x) -> p s c x", c=4, x=128)
                    return v[:, s0:s0 + n, c, :]

                # ---- finals: y=z-1, dr=1/(sum z - 9), out=z*dr-dr, out[C]=dr
                outq3 = mpool.tile([128, 4 * (C + 1)], f32, tag="outq3")

                def finals(m0, m1, outq):
                    n = m1 - m0
                    q = m0 // 4
                    szq = mpool.tile([128, n], f32, name=f"sz{m0}",
                                     tag=f"szq{n}", bufs=2)
                    nc.vector.tensor_reduce(
                        szq[:],
                        zf[:, C * m0:C * m1].rearrange("p (s k) -> p s k", k=C),
                        axis=mybir.AxisListType.X, op=Alu.add)
                    nc.vector.tensor_scalar_add(szq[:], szq[:], -(C - 1.0))
                    drq = mpool.tile([128, n], f32, name=f"dr{m0}",
                                     tag=f"drq{n}", bufs=2)
                    nc.vector.reciprocal(drq[:], szq[:])
                    for i in range(n):
                        s = m0 + i - 4 * q
                        nc.vector.tensor_scalar(
                            outq[:, (C + 1) * s:(C + 1) * s + C],
                            zf[:, C * (m0 + i):C * (m0 + i + 1)],
                            scalar1=drq[:, i:i + 1], scalar2=drq[:, i:i + 1],
                            op0=Alu.mult, op1=Alu.subtract)
                    ovv = outq[:].rearrange("p (s k) -> p s k", k=C + 1)
                    nc.gpsimd.tensor_copy(ovv[:, m0 - 4 * q:m1 - 4 * q, C],
                                          drq[:])

                def q_dma(q, outq):
                    nc.sync.dma_start(out_d[:, 44 * q:44 * (q + 1)], outq[:])

                # ---- chunk stage: qs matmul -> (+1/3) -> scan -> z extract
                def chunk_stage(m, rho, joff):
                    on_pe = m in ONPE
                    qs = pq.tile([128, QN], f32, name=f"qs{m}", tag="qs")
                    for (o, n) in nsplit:
                        nc.tensor.matmul(
                            qs[:, o:o + n],
                            rho[:, 128 * joff:128 * (joff + 1)],
                            ublk[:, o:o + n], start=True, stop=not on_pe)
                        if on_pe:
                            nc.tensor.matmul(
                                qs[:, o:o + n], ones2[0:1, :],
                                crow2[:, o:o + n], start=False, stop=False)
                            nc.tensor.matmul(
                                qs[:, o:o + n], ones2[0:1, :],
                                crow2[:, QN + o:QN + o + n],
                                start=False, stop=True)
                    so = spool.tile([128, QN], f32, name=f"so{m}", tag="so")
                    if on_pe:
                        data0 = qs
                    else:
                        sh = spool.tile([128, QN], f32, name=f"sh{m}", tag="sh")
                        nc.scalar.activation(sh[:], qs[:], Act.Copy,
                                             bias=1.0 / 3.0)
                        data0 = sh
                    nc.vector.tensor_tensor_scan(
                        so[:], data0[:], data1[:], initial=1.0,
                        op0=Alu.mult, op1=Alu.add)
                    sov = so[:].rearrange("p (k s) -> p k s", s=SEG)
                    nc.gpsimd.tensor_copy(
                        zf[:, C * m:C * (m + 1)], sov[:, :, SEG - 1])
                    # finals: batched per quarter; per chunk on last quarter
                    if m in (3, 7, 11):
                        q = m // 4
                        oq = mpool.tile([128, 4 * (C + 1)], f32,
                                        name=f"outq{q}", tag="outq", bufs=2)
                        finals(4 * q, 4 * q + 4, oq)
                        q_dma(q, oq)
                    elif m >= 12:
                        finals(m, m + 1, outq3)
                        if m == 15:
                            q_dma(3, outq3)

                # ---- per-slice pipeline (chunk stage lags one slice) ----
                pending = []          # (first_chunk, nchunks, rho_tile)
                for (cs, w) in SLICES:
                    T = pst.tile([128, 512], f32, name=f"T{cs}", tag="T")
                    for c in range(4):
                        nc.tensor.matmul(T[:, 0:w], wh[:, 128 * c:128 * (c + 1)],
                                         _xslice(cs, w, c),
                                         start=(c == 0), stop=False)
                    for c in range(4):
                        nc.tensor.matmul(T[:, 0:w],
                                         wh[:, 512 + 128 * c:512 + 128 * (c + 1)],
                                         _xslice(cs, w, c),
                                         start=False, stop=False)
                    nc.tensor.matmul(T[:, 0:w], ones2, cb2[:, cs:cs + w],
                                     start=False, stop=True)

                    si = mpool.tile([128, 512], f32, name=f"si{cs}", tag="si",
                                    bufs=3)
                    nc.scalar.activation(si[:, 0:w], T[:, 0:w], Act.Exp,
                                         bias=crowb[:, 0:1])
                    amax = mpool.tile([128, 512], f32, name=f"am{cs}",
                                      tag="amax", bufs=2)
                    nc.gpsimd.partition_all_reduce(
                        amax[:, 0:w], si[:, 0:w], channels=128,
                        reduce_op=bass_isa.ReduceOp.max)
                    d0 = spool.tile([128, 512], f32, name=f"d0{cs}", tag="d0")
                    nc.gpsimd.tensor_sub(d0[:, 0:w], amax[:, 0:w], si[:, 0:w])
                    dent = spool.tile([128, 512], f32, name=f"dent{cs}",
                                      tag="dent")
                    nc.scalar.activation(dent[:, 0:w], d0[:, 0:w], Act.Copy,
                                         bias=1e-4)
                    rec = mpool.tile([128, 512], f32, name=f"rec{cs}",
                                     tag="rec", bufs=2)
                    nc.vector.reciprocal_approx_fast(rec[:, 0:w], dent[:, 0:w])
                    rho = mpool.tile([128, 512], f32r, name=f"rho{cs}",
                                     tag="rho", bufs=4)
                    nc.gpsimd.tensor_mul(rho[:, 0:w], si[:, 0:w], rec[:, 0:w])
                    for (m0, nch, rr) in pending:
                        for j in range(nch):
                            chunk_stage(m0 + j, rr, j)
                    pending = [(cs // 128, w // 128, rho)]
                for (m0, nch, rr) in pending:
                    for j in range(nch):
                        chunk_stage(m0 + j, rr, j)

    nc.compile()
    return nc


def _f32r_round(v):
    # float32r = RNE to 11 explicit mantissa bits (HW-verified).
    u = np.asarray(v, np.float32).view(np.uint32).astype(np.uint64)
    drop = 12
    half = np.uint64(1 << (drop - 1))
    odd = (u >> np.uint64(drop)) & np.uint64(1)
    u2 = (u + half - np.uint64(1) + odd) & np.uint64(~((1 << drop) - 1) & 0xFFFFFFFF)
    return u2.astype(np.uint32).view(np.float32)


def _host_prep(x, w, eta, xi, beta):
    """Host-side: shard/layout x, build tiny replicated param matrices."""
    import ml_dtypes

    x = np.asarray(x, np.float32)
    w = np.asarray(w, np.float32)
    eta = np.asarray(eta, np.float32).reshape(-1)
    xi = np.asarray(xi, np.float32).reshape(-1)
    beta = np.asarray(beta, np.float32)

    gamma = (eta.astype(np.float64)) ** 2                # [P]
    if np.ptp(gamma) != 0.0:
        raise NotImplementedError(
            "kernel assumes per-prototype-constant gamma (eta); the shipped "
            "problem uses eta = full(0.1)")
    g0 = float(gamma[0])
    alpha = 1.0 / (1.0 + np.exp(-xi.astype(np.float64)))
    wsq = (w.astype(np.float64) ** 2).sum(-1)            # [P]

    wt2g = 2.0 * gamma[None, :] * w.T.astype(np.float64)   # [F,P] f64
    whb = wt2g.astype(np.float16)                        # [F, P] fp16 hi
    wrb = (wt2g - whb.astype(np.float64)).astype(np.float16)  # fp16 residual
    whp = np.zeros((128, 1024), np.float16)
    for c in range(4):
        whp[:, 128 * c:128 * (c + 1)] = whb[128 * c:128 * (c + 1), :]
        whp[:, 512 + 128 * c:512 + 128 * (c + 1)] = wrb[128 * c:128 * (c + 1), :]

    crow_bias = (np.log(alpha) - gamma * wsq).astype(np.float32)[:, None]  # [P,1]

    b2 = beta.astype(np.float64) ** 2
    u = b2 / b2.sum(0, keepdims=True)                    # [C,P]
    uh = u / 3.0
    third_a = float(_f32r_round(np.float32(1.0 / 3.0)))
    third_b = np.float32(1.0 / 3.0 - third_a)
    ublk = np.zeros((P, QN), np.float32)
    crow2ab = np.zeros((1, 2 * QN), np.float32)
    for k in range(C):
        base = k * SEG
        crow2ab[0, base + 1:base + SEG] = third_a
        crow2ab[0, QN + base + 1:QN + base + SEG] = third_b
        for t in range(P):
            v = uh[k, t] * (3.0 if t == 0 else 1.0)
            ublk[t, base + 1 + t] = np.float32(v)

    shards = x.reshape(NCORES, BL, F)
    in_maps = []
    for i in range(NCORES):
        xs = shards[i]                                    # [BL, F] f32
        xt = np.ascontiguousarray(xs.T)                   # [F, BL]
        xh = xt.astype(np.float16)
        # slice-contiguous: block (slice s of 16, chunk c) at col s*512+c*128
        xqp = np.zeros((128, 8192), np.float16)
        for s in range(16):
            for c in range(4):
                xqp[:, 512 * s + 128 * c:512 * s + 128 * (c + 1)] = \
                    xh[128 * c:128 * (c + 1), 128 * s:128 * (s + 1)]
        # -g|x|^2 per batch row, f64 -> f32r a + f32r b residual rows
        cb = -(g0 * (xs.astype(np.float64) ** 2).sum(-1))          # [BL]
        cba = _f32r_round(cb.astype(np.float32))
        cbb = _f32r_round((cb - cba.astype(np.float64)).astype(np.float32))
        cb2 = np.stack([cba, cbb], axis=0)                         # [2, BL]
        in_maps.append({
            "xq": xqp, "whp": whp, "cb2": cb2, "crowb": crow_bias,
            "ublk": ublk, "crow2ab": crow2ab,
        })
    return in_maps


def _run(in_maps, trace=False):
    from concourse.bass_utils import run_bass_kernel_spmd

    if "nc" not in _PROG:
        _PROG["nc"] = _build_program()
    nc = _PROG["nc"]
    res = run_bass_kernel_spmd(
        nc, in_maps, core_ids=list(range(NCORES)), trace=trace)
    outs = []
    for i in range(NCORES):
        o = np.asarray(res.results[i]["out"])          # [128, 176]
        outs.append(o.reshape(128, 16, C + 1).transpose(1, 0, 2).reshape(BL, C + 1))
    full = np.concatenate(outs, axis=0).astype(np.float32)
    return full, res


def kernel(x, w, eta, xi, beta):
    in_maps = _host_prep(x, w, eta, xi, beta)
    full, _ = _run(in_maps, trace=False)
    return full
